# revision 40
# baseline (speedup 1.0000x reference)
"""MultiHeadCrossAttention on 8 TRN2 NeuronCores.

Sharding: core c -> batch b = c//2, head-group g = c%2 (8 heads, 512 out dims).
Each core computes its head-group's Q/K/V projections, attention, and a
partial out-projection (Wo columns restricted to its head-group). Host sums
the two partials per batch and adds bo.

Softmax denominator trick: V is augmented with a ones-column per head
(via a zero weight column + bias 1.0), so attn@V_aug row 64 of each head's
65-wide block is the softmax denominator; normalize with DVE reciprocal +
a K=1 matmul broadcast.

All stream DMAs rotate through one shared pool with bufs == NUM_HWDGE_SEMS
(8) so slot reuse lands on the same DMA lane (WAW collapses into queue
order) and every DMA carries at most 2 semaphore waits (walrus limit).
"""

import sys

import numpy as np

if "/opt/trn_rl_repo" not in sys.path:
    sys.path.insert(0, "/opt/trn_rl_repo")

import concourse.bacc as bacc
import concourse.bass as bass
import concourse.mybir as mybir
import concourse.tile as tile
from concourse.bass_utils import run_bass_kernel_spmd

FP32 = mybir.dt.float32
FP32R = mybir.dt.float32r
FP16 = mybir.dt.float16

B, NQ, NK = 4, 1024, 2048
QD, KD = 1024, 768
H, D = 16, 64
E = H * D  # 1024 total embed dim
G = 8  # heads per core
GO = G * D  # 512 out dims per core
DA = D + 1  # 65: head dim + denominator column
GA = G * DA  # 520
SCALE = 1.0 / 8.0

USE_FP32R = True
MMDT = FP32R if USE_FP32R else FP32

# test.py hooks
TRACE = False
TRACE_KWARGS = {}
LAST_RESULT = None


def _mm(nc, out, lhsT, rhs, start, stop):
    nc.tensor.matmul(out, lhsT, rhs, start=start, stop=stop)


def build_program():
    nc = bacc.Bacc()

    qT = nc.declare_dram_parameter("qT", [QD, NQ], FP16, isOutput=False)
    kT = nc.declare_dram_parameter("kT", [KD, NK], FP16, isOutput=False)
    vT = nc.declare_dram_parameter("vT", [KD, NK], FP16, isOutput=False)
    wq = nc.declare_dram_parameter("wq", [QD, GO], FP16, isOutput=False)
    wk = nc.declare_dram_parameter("wk", [KD, GO], FP16, isOutput=False)
    wv = nc.declare_dram_parameter("wv", [KD, 1024], FP16, isOutput=False)
    wo = nc.declare_dram_parameter("wo", [GO, E], MMDT, isOutput=False)
    vbias = nc.declare_dram_parameter("vbias", [128, 1024], FP32, isOutput=False)
    bq = nc.declare_dram_parameter("bq", [128, 4], FP32, isOutput=False)
    bk = nc.declare_dram_parameter("bk", [128, 4], FP32, isOutput=False)
    out = nc.declare_dram_parameter("out", [NQ, E], FP32, isOutput=True)

    with (
        nc.allow_low_precision("fp16 attention activations; validated 1.7e-4 rel"),
        tile.TileContext(nc) as tc,
    ):
        with (
            tc.tile_pool(name="consts", bufs=1) as consts,
            tc.tile_pool(name="wo_p", bufs=1) as wo_p,
            tc.tile_pool(name="qt_p", bufs=1) as qt_p,
            tc.tile_pool(name="kt_p", bufs=1) as kt_p,
            tc.tile_pool(name="va_p", bufs=1) as va_p,
            tc.tile_pool(name="osb_p", bufs=1) as osb_p,
            tc.tile_pool(name="strm_p", bufs=1) as strm_p,
        ):
            vbias_sb = consts.tile([128, 1024], FP32)
            nc.sync.dma_start(vbias_sb[:], vbias[:, :])
            bq_sb = consts.tile([128, 4], FP32)
            nc.sync.dma_start(bq_sb[:], bq[:, :])
            bk_sb = consts.tile([128, 4], FP32)
            nc.sync.dma_start(bk_sb[:], bk[:, :])
            ones_sb = consts.tile([1, 64], FP16)
            nc.vector.memset(ones_sb[:], 1.0)

            wo_sb = []
            for kk in range(4):
                t = wo_p.tile([128, E], MMDT, name=f"wo{kk}")
                nc.sync.dma_start(t[:], wo[kk * 128 : (kk + 1) * 128, :])
                wo_sb.append(t)

            # Persistent activation tiles.
            # Qt[m][n]: [128 outdim, 512 tokq]   (m: outdim tile, n: tokq chunk)
            qt_sb = [
                [qt_p.tile([128, 512], FP16, name=f"qt{m}_{n}") for n in range(2)]
                for m in range(4)
            ]
            # Kt zero-padded per head-half: ktz[m][hl][c] is [128 outdim, 512 tokk]
            # where only partitions [hl*64, hl*64+64) hold data, the rest are 0.
            # Full-128-partition lhsT keeps the PE at 1 cyc/col (K=64 runs 2.5x
            # slower on real HW regardless of dtype).
            ktz = [
                [
                    [
                        kt_p.tile([128, 512], FP16, name=f"ktz{m}_{hl}_{c}")
                        for c in range(4)
                    ]
                    for hl in range(2)
                ]
                for m in range(4)
            ]
            for m in range(4):
                for c in range(4):
                    nc.vector.memset(ktz[m][0][c][64:128, :], 0.0)
                    nc.vector.memset(ktz[m][1][c][0:64, :], 0.0)
            # V_aug[t]: [128 tokk, 1024]  (per-head 128-col blocks: 64 V dims,
            # col 64 = ones/denominator, cols 65-127 zero pad so PV runs M=128)
            va_sb = [va_p.tile([128, 1024], FP16, name=f"va{t}") for t in range(16)]
            # O^T (normalized) [concat dim 512 -> 4 tiles of 128, tokq 1024]
            osb = [osb_p.tile([128, NQ], MMDT, name=f"osb{t}") for t in range(4)]

            # fp16 stream buffers for qT/kT/vT (12 in flight keeps the 16 DMA
            # queues fed; slot reuse dependency chains are the DMA pacing item).
            strm_tiles = [
                strm_p.tile([128, 512], FP16, name=f"strm{i}") for i in range(16)
            ]
            strm_ctr = [0]

            def strm():
                t = strm_tiles[strm_ctr[0] % 16]
                strm_ctr[0] += 1
                return t

            # ---- Phases B-D share the projection-weight scratch scope ----
            with (
                tc.tile_pool(name="wq_p", bufs=1) as wq_p,
                tc.tile_pool(name="wk_p", bufs=1) as wk_p,
                tc.tile_pool(name="wv_p", bufs=1) as wv_p,
            ):
                wq_sb = []
                for kk in range(8):
                    t = wq_p.tile([128, GO], FP16, name=f"wq{kk}")
                    nc.sync.dma_start(t[:], wq[kk * 128 : (kk + 1) * 128, :])
                    wq_sb.append(t)
                wk_sb = []
                for kk in range(6):
                    t = wk_p.tile([128, GO], FP16, name=f"wk{kk}")
                    nc.sync.dma_start(t[:], wk[kk * 128 : (kk + 1) * 128, :])
                    wk_sb.append(t)
                wv_sb = []
                for kk in range(6):
                    t = wv_p.tile([128, 1024], FP16, name=f"wv{kk}")
                    nc.sync.dma_start(t[:], wv[kk * 128 : (kk + 1) * 128, :])
                    wv_sb.append(t)

                # ---- Phase B: Q projection. Qt = Wq_g @ query^T (+bq) ----
                with tc.tile_pool(name="psB", bufs=1, space="PSUM") as psB:
                    psq = [
                        [
                            psB.tile([128, 512], FP32, name=f"psq{m}_{n}")
                            for n in range(2)
                        ]
                        for m in range(4)
                    ]
                    for kk in range(8):
                        for n in range(2):
                            qs = strm()
                            nc.sync.dma_start(
                                qs[:],
                                qT[kk * 128 : (kk + 1) * 128, n * 512 : (n + 1) * 512],
                            )
                            for m in range(4):
                                _mm(
                                    nc,
                                    psq[m][n][:],
                                    wq_sb[kk][:, m * 128 : (m + 1) * 128],
                                    qs[:],
                                    start=(kk == 0),
                                    stop=(kk == 7),
                                )
                    for m in range(4):
                        for n in range(2):
                            nc.vector.tensor_scalar_add(
                                qt_sb[m][n][:], psq[m][n][:], bq_sb[:, m : m + 1]
                            )

                # ---- Phase C: K projection. Kt = Wk_g @ key^T (+bk) ----
                with tc.tile_pool(name="psC", bufs=1, space="PSUM") as psC:
                    for half in range(2):
                        psk = [
                            [
                                psC.tile([128, 512], FP32, name=f"psk{m}_{n}")
                                for n in range(2)
                            ]
                            for m in range(4)
                        ]
                        for kk in range(6):
                            for n in range(2):
                                ks_ = strm()
                                c0 = half * 1024 + n * 512
                                nc.sync.dma_start(
                                    ks_[:],
                                    kT[kk * 128 : (kk + 1) * 128, c0 : c0 + 512],
                                )
                                for m in range(4):
                                    _mm(
                                        nc,
                                        psk[m][n][:],
                                        wk_sb[kk][:, m * 128 : (m + 1) * 128],
                                        ks_[:],
                                        start=(kk == 0),
                                        stop=(kk == 5),
                                    )
                        for m in range(4):
                            for n in range(2):
                                c = half * 2 + n
                                nc.vector.tensor_scalar_add(
                                    ktz[m][0][c][0:64, :],
                                    psk[m][n][0:64, :],
                                    bk_sb[0:64, m : m + 1],
                                )
                                nc.vector.tensor_scalar_add(
                                    ktz[m][1][c][64:128, :],
                                    psk[m][n][64:128, :],
                                    bk_sb[64:128, m : m + 1],
                                )

                # ---- Phase D: V_aug = value @ Wv_aug^T (+vbias, ones col) ----
                with tc.tile_pool(name="psD", bufs=1, space="PSUM") as psD:
                    for tb in range(4):
                        psv = [
                            [
                                psD.tile([128, 512], FP32, name=f"psv{t2}_{n}")
                                for n in range(2)
                            ]
                            for t2 in range(4)
                        ]
                        for kk in range(6):
                            vs = strm()
                            nc.sync.dma_start(
                                vs[:],
                                vT[kk * 128 : (kk + 1) * 128, tb * 512 : (tb + 1) * 512],
                            )
                            for t2 in range(4):
                                for n in range(2):
                                    _mm(
                                        nc,
                                        psv[t2][n][:],
                                        vs[:, t2 * 128 : (t2 + 1) * 128],
                                        wv_sb[kk][:, n * 512 : (n + 1) * 512],
                                        start=(kk == 0),
                                        stop=(kk == 5),
                                    )
                        for t2 in range(4):
                            for n in range(2):
                                nc.vector.tensor_add(
                                    va_sb[tb * 4 + t2][:, n * 512 : (n + 1) * 512],
                                    psv[t2][n][:],
                                    vbias_sb[:, n * 512 : (n + 1) * 512],
                                )

            # ---- Phase E: attention per head ----
            # Exp grouped into [128,1024] 2-bank ACT calls (halves per-call
            # overhead). Normalize chain (recip -> K=1 broadcast mm -> mul) is
            # deferred by one (h,n) iteration so the 3.4us DVE reciprocal never
            # stalls the in-order PE queue.
            with (
                tc.tile_pool(name="otp", bufs=3, space="PSUM") as otp,
                tc.tile_pool(name="stp", bufs=2, space="PSUM") as stp,
                tc.tile_pool(name="bcp", bufs=1, space="PSUM") as bcp,
                tc.tile_pool(name="p_p", bufs=4) as p_p,
                tc.tile_pool(name="rc_p", bufs=2) as rc_p,
                tc.tile_pool(name="bcs_p", bufs=2) as bcs_p,
            ):
                iters = [(h, n) for h in range(G) for n in range(2)]
                pending = [None]
                ot_cur = [None]

                def flush_pending():
                    ot_p, rc_t, mt_p, po_p, n_p = pending[0]
                    bc_t = bcp.tile([128, 512], FP32, name="bc")
                    nc.tensor.matmul(
                        bc_t[:64, :], ones_sb[:, :], rc_t[:], start=True, stop=True
                    )
                    bcs = bcs_p.tile([64, 512], FP16, name="bcs")
                    nc.vector.tensor_copy(bcs[:], bc_t[:64, :])
                    nc.vector.tensor_mul(
                        osb[mt_p][po_p : po_p + 64, n_p * 512 : (n_p + 1) * 512],
                        ot_p[:64, :],
                        bcs[:],
                    )
                    pending[0] = None

                def do_pv(pit, pg, pp):
                    ph, pn = iters[pit]
                    if pg == 0:
                        ot_cur[0] = otp.tile([128, 512], FP32, name="ot")
                    ot = ot_cur[0]
                    for j in range(2):
                        _mm(
                            nc,
                            ot[:],
                            va_sb[2 * pg + j][:, ph * 128 : (ph + 1) * 128],
                            pp[:, j * 512 : (j + 1) * 512],
                            start=(pg == 0 and j == 0),
                            stop=(pg == 7 and j == 1),
                        )
                    if pg == 7:
                        if pending[0] is not None:
                            flush_pending()
                        rc = rc_p.tile([1, 512], FP16, name="rc")
                        nc.vector.reciprocal(rc[:], ot[64:65, :])
                        pending[0] = (ot, rc, ph // 2, (ph % 2) * 64, pn)

                queue = []
                for it in range(16):
                    h, n = iters[it]
                    mt, hl = h // 2, h % 2
                    for g2 in range(8):
                        st2 = stp.tile([128, 1024], FP32, name="st")
                        for j in range(2):
                            kt = 2 * g2 + j
                            _mm(
                                nc,
                                st2[:, j * 512 : (j + 1) * 512],
                                ktz[mt][hl][kt // 4][
                                    :, (kt % 4) * 128 : (kt % 4 + 1) * 128
                                ],
                                qt_sb[mt][n][:],
                                start=True,
                                stop=True,
                            )
                        p2 = p_p.tile([128, 1024], FP16, name="p")
                        nc.scalar.activation(
                            p2[:],
                            st2[:],
                            mybir.ActivationFunctionType.Exp,
                            bias=0.0,
                            scale=SCALE,
                        )
                        queue.append((it, g2, p2))
                        if len(queue) >= 2:
                            do_pv(*queue.pop(0))
                do_pv(*queue.pop(0))
                flush_pending()

            # ---- Phase F: partial out projection Y_part = O_g @ Wo[:, g]^T ----
            with (
                tc.tile_pool(name="psF", bufs=4, space="PSUM") as psF,
                tc.tile_pool(name="ys_p", bufs=4) as ys_p,
            ):
                for m in range(8):
                    for n in range(2):
                        psy = psF.tile([128, 512], FP32, name="psy")
                        for kt in range(4):
                            _mm(
                                nc,
                                psy[:],
                                osb[kt][:, m * 128 : (m + 1) * 128],
                                wo_sb[kt][:, n * 512 : (n + 1) * 512],
                                start=(kt == 0),
                                stop=(kt == 3),
                            )
                        ys = ys_p.tile([128, 512], FP32, name="ys")
                        nc.vector.tensor_copy(ys[:], psy[:])
                        nc.sync.dma_start(
                            out[m * 128 : (m + 1) * 128, n * 512 : (n + 1) * 512],
                            ys[:],
                        )

    nc.finalize()
    return nc


def kernel(**inputs):
    global LAST_RESULT
    arrs = {k: np.asarray(v, dtype=np.float32) for k, v in inputs.items()}
    query, key, value = arrs["query"], arrs["key"], arrs["value"]
    Wq, bq_, Wk, bk_ = arrs["Wq"], arrs["bq"], arrs["Wk"], arrs["bk"]
    Wv, bv_, Wo, bo_ = arrs["Wv"], arrs["bv"], arrs["Wo"], arrs["bo"]

    nc = build_program()

    qTb = [np.ascontiguousarray(query[b].T.astype(np.float16)) for b in range(B)]
    kTb = [np.ascontiguousarray(key[b].T.astype(np.float16)) for b in range(B)]
    vTb = [np.ascontiguousarray(value[b].T.astype(np.float16)) for b in range(B)]

    per_group = []
    for g in range(2):
        gs = slice(g * GO, (g + 1) * GO)
        wq_m = np.ascontiguousarray(Wq[gs, :].T.astype(np.float16))
        wk_m = np.ascontiguousarray(Wk[gs, :].T.astype(np.float16))
        wv_aug = np.zeros((KD, 1024), np.float32)
        vb_row = np.zeros((1024,), np.float32)
        for h in range(G):
            hs = slice(g * GO + h * D, g * GO + (h + 1) * D)
            wv_aug[:, h * 128 : h * 128 + D] = Wv[hs, :].T
            vb_row[h * 128 : h * 128 + D] = bv_[hs]
            vb_row[h * 128 + D] = 1.0
        vbias_m = np.ascontiguousarray(np.tile(vb_row, (128, 1)).astype(np.float32))
        wo_m = np.ascontiguousarray(Wo[:, gs].T)
        bq_m = np.ascontiguousarray(bq_[gs].reshape(4, 128).T)
        bk_m = np.ascontiguousarray(bk_[gs].reshape(4, 128).T)
        per_group.append(
            {
                "wq": wq_m,
                "wk": wk_m,
                "wv": wv_aug.astype(np.float16),
                "wo": wo_m,
                "vbias": vbias_m,
                "bq": bq_m,
                "bk": bk_m,
            }
        )

    in_maps = []
    for c in range(8):
        b, g = c // 2, c % 2
        m = {"qT": qTb[b], "kT": kTb[b], "vT": vTb[b]}
        m.update(per_group[g])
        in_maps.append(m)

    res = run_bass_kernel_spmd(
        nc, in_maps, list(range(8)), trace=TRACE, **(TRACE_KWARGS if TRACE else {})
    )
    LAST_RESULT = res

    outs = res.results
    Y = np.empty((B, NQ, E), np.float32)
    for b in range(B):
        Y[b] = outs[2 * b]["out"] + outs[2 * b + 1]["out"] + bo_[None, :]
    return Y
